# revision 13
# baseline (speedup 1.0000x reference)
"""Perlin noise (32,512,512) volumetric grid on 8 Trainium2 NeuronCores.

Strategy
--------
Data-parallel: shard the 8.4M query points across 8 cores (1,048,576 each,
laid out [128, 8192] in SBUF tiles, processed in chunks).

The hash chain  perm[(perm[(perm[xi]+yi)&255]+zi)&255] % 12  is precomputed
on the host into small fused lookup tables, and evaluated per-element with
GPSIMD ap_gather (the only per-element-indexed primitive on TRN2):
  * TB4[q], q = 64*xi + yi (4096 rows x 4 uint8, AoS): the four level-2
    values B_XY = perm[(perm[xi+X] + yi + Y) & 255], X,Y in {0,1}.
    One d=4 uint8 gather (a single 4-byte word per index) fetches all four.
  * TG[k], k = B_XY + zi in [0,318] (320 rows, AoS, mod-256 folded into
    the table so no on-device mod is needed): gradient components
    grad3[perm[k]%12][:] and grad3[perm[(k+1)&255]%12][:] -- one gather per
    (X,Y) pair fetches all six components for both z-corners.  Stored as
    fp8e4m3 (d=8, 2-word rows) when grad3 is exactly representable (true for
    the canonical +-1/0 table), else bf16 (d=6).
ap_gather's output order interleaves the 16 partitions of each GPSIMD core
(position i = s*16+p), so small per-sublane DMAs re-pack the AoS rows back
into element-aligned tiles.  Dense math (floor/fade/dot/trilerp) is fp32 on
DVE/ACT, overlapped with the gathers by the Tile framework.
"""

import numpy as np

GRID = (32, 512, 512)
NTOT = GRID[0] * GRID[1] * GRID[2]
NCORES = 8
NC_ELEMS = NTOT // NCORES          # 1,048,576 per core
ROWS = 128
COLS = NC_ELEMS // ROWS            # 8192
F = 256                            # chunk free-dim
NCHUNK = COLS // F

_cache = {}


def _to_bf16(a):
    import ml_dtypes
    return a.astype(ml_dtypes.bfloat16)


def _build_tables(perm, grad3):
    import ml_dtypes
    perm = np.asarray(perm).astype(np.int64)
    grad3 = np.asarray(grad3).astype(np.float32)
    XI, YI = np.meshgrid(np.arange(64), np.arange(64), indexing="ij")
    A0 = perm[XI & 255]
    A1 = perm[(XI + 1) & 255]
    tb = np.stack([
        perm[(A0 + YI) & 255],
        perm[(A0 + YI + 1) & 255],
        perm[(A1 + YI) & 255],
        perm[(A1 + YI + 1) & 255],
    ], axis=-1).reshape(4096 * 4).astype(np.uint8)        # AoS, q-major

    k = np.arange(320)
    gi0 = perm[k & 255] % 12
    gi1 = perm[(k + 1) & 255] % 12
    tg6 = np.stack([
        grad3[gi0, 0], grad3[gi1, 0],
        grad3[gi0, 1], grad3[gi1, 1],
        grad3[gi0, 2], grad3[gi1, 2],
    ], axis=-1)                                           # [320, 6], k-major

    TB = np.broadcast_to(tb, (128, 4096 * 4)).copy()

    # gradient table: fp8 (d=8, 2-word rows -> cheaper gather) when exact,
    # else bf16 (d=6, 3-word rows)
    g_fp8 = bool(
        np.array_equal(
            grad3.astype(ml_dtypes.float8_e4m3).astype(np.float32), grad3))
    if g_fp8:
        tg8 = np.zeros((320, 8), np.float32)
        tg8[:, :6] = tg6
        TG = np.broadcast_to(
            tg8.reshape(-1).astype(ml_dtypes.float8_e4m3), (128, 320 * 8)).copy()
    else:
        TG = np.broadcast_to(
            _to_bf16(tg6.reshape(-1)), (128, 320 * 6)).copy()
    return TB, TG, g_fp8


def _build_nc(nchunk, g_fp8):
    import sys
    if "/opt/trn_rl_repo" not in sys.path:
        sys.path.insert(0, "/opt/trn_rl_repo")
    import concourse.bacc as bacc
    import concourse.tile as tile
    import concourse.mybir as mybir

    dt = mybir.dt
    op = mybir.AluOpType
    nc = bacc.Bacc("TRN2", target_bir_lowering=False, debug=False)

    for val in (10.0, -1.0):
        t = nc.alloc_sbuf_tensor(f"const-f32-{val}", [128, 1], dt.float32)
        nc.gpsimd.memset(t.ap(), val)
        nc.const_aps.aps[(dt.float32, val)] = t.ap()
    nc.all_engine_barrier()

    cols = nchunk * F
    xd = nc.dram_tensor("x", [ROWS, cols], dt.float32, kind="ExternalInput")
    yd = nc.dram_tensor("y", [ROWS, cols], dt.float32, kind="ExternalInput")
    zd = nc.dram_tensor("z", [ROWS, cols], dt.float32, kind="ExternalInput")
    tbd = nc.dram_tensor("TB", [ROWS, 4096 * 4], dt.uint8, kind="ExternalInput")
    gdt = dt.float8e4 if g_fp8 else dt.bfloat16
    gd = 8 if g_fp8 else 6
    tgd = nc.dram_tensor("TG", [ROWS, 320 * gd], gdt, kind="ExternalInput")
    outd = nc.dram_tensor("out", [ROWS, cols], dt.float32, kind="ExternalOutput")

    dma_engines = None

    with tile.TileContext(nc) as tc:
        dma_engines = [nc.sync, nc.scalar]

        with tc.tile_pool(name="const", bufs=1) as cpool, \
             tc.tile_pool(name="gbig", bufs=1) as gpool, \
             tc.tile_pool(name="work", bufs=2) as pool:
            tb = cpool.tile([128, 4096 * 4], dt.uint8)
            tg = cpool.tile([128, 320 * gd], gdt)
            nc.sync.dma_start(tb[:], tbd.ap())
            nc.sync.dma_start(tg[:], tgd.ap())

            def lerp(dst, a, b, t, tmp):
                nc.vector.tensor_tensor(tmp[:], b[:], a[:], op.subtract)
                nc.vector.tensor_tensor(tmp[:], tmp[:], t[:], op.mult)
                nc.vector.tensor_tensor(dst[:], a[:], tmp[:], op.add)

            def unbraid_half(src_big, d, dst, h, H, eng_off=0):
                # like unbraid, for the h-th half of dst (cols h*H..h*H+H)
                s = src_big[:].rearrange("(k g) (f p d) -> k g f p d",
                                         g=16, p=16, d=d)
                t_ = dst[:].rearrange("(k g) (f d) -> k g f d", g=16, d=d)
                for p in range(16):
                    eng = dma_engines[(p + eng_off) % len(dma_engines)]
                    eng.dma_start(t_[:, p, h * H:(h + 1) * H, :],
                                  s[:, p, :, p, :])

            def unbraid(src_big, d, dst, eng_off=0):
                # src_big [128, 16F*d] gather AoS out (pos i = f*16+p);
                # dst [128, F*d] with element (16k+p, f) comps at f*d+j.
                s = src_big[:].rearrange("(k g) (f p d) -> k g f p d",
                                         g=16, p=16, d=d)
                t_ = dst[:].rearrange("(k g) (f d) -> k g f d", g=16, d=d)
                for p in range(16):
                    eng = dma_engines[(p + eng_off) % len(dma_engines)]
                    eng.dma_start(t_[:, p, :, :], s[:, p, :, p, :])

            def phase_a(ch):
                """loads, floors, q-index, B-gather (emitted one chunk ahead
                so the B-gather fills GPSIMD stalls of the previous chunk)."""
                sl = slice(ch * F, (ch + 1) * F)
                tx = pool.tile([128, F], dt.float32, tag="tx")
                ty = pool.tile([128, F], dt.float32, tag="ty")
                tz = pool.tile([128, F], dt.float32, tag="tz")
                nc.sync.dma_start(tx[:], xd.ap()[:, sl])
                nc.sync.dma_start(ty[:], yd.ap()[:, sl])
                nc.sync.dma_start(tz[:], zd.ap()[:, sl])

                fx = pool.tile([128, F], dt.float32, tag="fx")
                fy = pool.tile([128, F], dt.float32, tag="fy")
                fz = pool.tile([128, F], dt.float32, tag="fz")
                xf = pool.tile([128, F], dt.float32, tag="xf")
                yf = pool.tile([128, F], dt.float32, tag="yf")
                zf = pool.tile([128, F], dt.float32, tag="zf")
                for (tin, fl, fr, tg_) in ((tx, fx, xf, "x"), (ty, fy, yf, "y"),
                                           (tz, fz, zf, "z")):
                    ti = pool.tile([128, F], dt.int32, tag="ti" + tg_)
                    nc.vector.tensor_copy(ti[:], tin[:])       # f32 -> i32 (rounds)
                    nc.vector.tensor_copy(fl[:], ti[:])        # i32 -> f32
                    # floor fixup: fl -= (fl > x)
                    gt = pool.tile([128, F], dt.float32, tag="gt" + tg_)
                    nc.vector.tensor_tensor(gt[:], fl[:], tin[:], op.is_gt)
                    nc.vector.tensor_tensor(fl[:], fl[:], gt[:], op.subtract)
                    nc.vector.tensor_tensor(fr[:], tin[:], fl[:], op.subtract)

                qf = pool.tile([128, F], dt.float32, tag="qf")
                nc.scalar.mul(qf[:], fx[:], 64.0)
                nc.vector.tensor_tensor(qf[:], qf[:], fy[:], op.add)
                qi = pool.tile([128, F], dt.int16, tag="qi")
                nc.vector.tensor_copy(qi[:], qf[:])
                bq = pool.tile([128, 16 * F * 4], dt.uint8, tag="bq")
                nc.gpsimd.ap_gather(bq[:], tb[:], qi[:],
                                    channels=128, num_elems=4096, d=4,
                                    num_idxs=16 * F)
                return dict(xf=xf, yf=yf, zf=zf, fz=fz, bq=bq)

            def phase_b(ch, st):
                sl = slice(ch * F, (ch + 1) * F)
                xf, yf, zf, fz, bq = st["xf"], st["yf"], st["zf"], st["fz"], st["bq"]
                bpl = pool.tile([128, F * 4], dt.uint8, tag="bpl")
                unbraid(bq, 4, bpl)
                bview = bpl[:].rearrange("p (f d) -> p f d", d=4)

                fades = []
                for t_, nm in ((xf, "u"), (yf, "v"), (zf, "w")):
                    p1 = pool.tile([128, F], dt.float32, tag="p1" + nm)
                    nc.vector.tensor_scalar(p1[:], t_[:], 6.0, -15.0, op.mult, op.add)
                    nc.vector.tensor_tensor(p1[:], p1[:], t_[:], op.mult)
                    nc.scalar.add(p1[:], p1[:], 10.0)
                    t2 = pool.tile([128, F], dt.float32, tag="t2" + nm)
                    nc.scalar.square(t2[:], t_[:])
                    nc.vector.tensor_tensor(t2[:], t2[:], t_[:], op.mult)
                    uu = pool.tile([128, F], dt.float32, tag=nm)
                    nc.vector.tensor_tensor(uu[:], t2[:], p1[:], op.mult)
                    fades.append(uu)
                u, v, w = fades

                xm = pool.tile([128, F], dt.float32, tag="xm")
                ym = pool.tile([128, F], dt.float32, tag="ym")
                zm = pool.tile([128, F], dt.float32, tag="zm")
                nc.scalar.add(xm[:], xf[:], -1.0)
                nc.scalar.add(ym[:], yf[:], -1.0)
                nc.scalar.add(zm[:], zf[:], -1.0)

                H = F // 2
                dots = {}
                for c in range(4):  # c = 2*X + Y
                    kf = pool.tile([128, F], dt.float32, tag="kf")
                    # k = B + zi in [0, 318]; TG has 320 rows so no mod
                    nc.vector.tensor_tensor(kf[:], bview[:, :, c], fz[:], op.add)
                    ki = pool.tile([128, F], dt.int16, tag="ki")
                    nc.vector.tensor_copy(ki[:], kf[:])
                    gpl = pool.tile([128, F * gd], gdt, tag=f"gpl{c}")
                    # two half-gathers ping-pong two small output buffers so
                    # gather h+1 overlaps the unbraid DMAs of gather h
                    for h in range(2):
                        gq = gpool.tile([128, 16 * H * gd], gdt, tag="gq")
                        nc.gpsimd.ap_gather(gq[:], tg[:], ki[:, h * H:(h + 1) * H],
                                            channels=128, num_elems=320, d=gd,
                                            num_idxs=16 * H)
                        unbraid_half(gq, gd, gpl, h, H, eng_off=c + h)
                    gv = gpl[:].rearrange("p (f d) -> p f d", d=gd)
                    dX = xm if (c >> 1) else xf
                    dY = ym if (c & 1) else yf
                    for zz, dZ in ((0, zf), (1, zm)):
                        acc = pool.tile([128, F], dt.float32, tag=f"d{c}{zz}")
                        tmp = pool.tile([128, F], dt.float32, tag="tmp")
                        nc.vector.tensor_tensor(acc[:], gv[:, :, 0 + zz], dX[:], op.mult)
                        nc.vector.tensor_tensor(tmp[:], gv[:, :, 2 + zz], dY[:], op.mult)
                        nc.vector.tensor_tensor(acc[:], acc[:], tmp[:], op.add)
                        nc.vector.tensor_tensor(tmp[:], gv[:, :, 4 + zz], dZ[:], op.mult)
                        nc.vector.tensor_tensor(acc[:], acc[:], tmp[:], op.add)
                        dots[(c, zz)] = acc

                # trilinear interpolation (matches reference lerp order)
                lt = pool.tile([128, F], dt.float32, tag="lt")
                x1 = pool.tile([128, F], dt.float32, tag="x1")
                x2 = pool.tile([128, F], dt.float32, tag="x2")
                x3 = pool.tile([128, F], dt.float32, tag="x3")
                x4 = pool.tile([128, F], dt.float32, tag="x4")
                lerp(x1, dots[(0, 0)], dots[(2, 0)], u, lt)
                lerp(x2, dots[(1, 0)], dots[(3, 0)], u, lt)
                lerp(x3, dots[(0, 1)], dots[(2, 1)], u, lt)
                lerp(x4, dots[(1, 1)], dots[(3, 1)], u, lt)
                y1 = pool.tile([128, F], dt.float32, tag="y1")
                y2 = pool.tile([128, F], dt.float32, tag="y2")
                lerp(y1, x1, x2, v, lt)
                lerp(y2, x3, x4, v, lt)
                res = pool.tile([128, F], dt.float32, tag="res")
                lerp(res, y1, y2, w, lt)
                nc.sync.dma_start(outd.ap()[:, sl], res[:])

            # software pipeline: B-gather of chunk ch+1 is queued on GPSIMD
            # before the gradient gathers of chunk ch, hiding the
            # unbraid/kidx latency between them.
            st = phase_a(0)
            for ch in range(nchunk):
                st_next = phase_a(ch + 1) if ch + 1 < nchunk else None
                phase_b(ch, st)
                st = st_next

    nc.compile()
    return nc


def kernel(**inputs):
    import sys
    if "/opt/trn_rl_repo" not in sys.path:
        sys.path.insert(0, "/opt/trn_rl_repo")
    from concourse import bass_utils

    x = np.asarray(inputs["x"], np.float32).reshape(-1)
    y = np.asarray(inputs["y"], np.float32).reshape(-1)
    z = np.asarray(inputs["z"], np.float32).reshape(-1)
    TB, TG, g_fp8 = _build_tables(inputs["perm"], inputs["grad3"])

    key = ("nc", g_fp8)
    if key not in _cache:
        _cache[key] = _build_nc(NCHUNK, g_fp8)
    nc = _cache[key]

    in_maps = []
    for i in range(NCORES):
        sl = slice(i * NC_ELEMS, (i + 1) * NC_ELEMS)
        in_maps.append({
            "x": x[sl].reshape(ROWS, COLS),
            "y": y[sl].reshape(ROWS, COLS),
            "z": z[sl].reshape(ROWS, COLS),
            "TB": TB,
            "TG": TG,
        })
    res = bass_utils.run_bass_kernel_spmd(nc, in_maps, core_ids=list(range(NCORES)))
    out = np.concatenate([res.results[i]["out"].reshape(-1) for i in range(NCORES)])
    return out.reshape(GRID).astype(np.float32)
